# revision 15
# baseline (speedup 1.0000x reference)
"""Trainium2 Bass kernel for ConditionalLoRALinear.

Reference computation (f32):
    base = x @ W.T + b                      # [B,S,Do]
    lora = (x @ A.T) @ B.T * 2.0            # rank-8
    out  = base + lora * (ids == 7)         # per-token gate

Sharding over 8 NeuronCores: 2 token-halves x 4 d_out-quarters.

Main loop: k-chunks 0..29 run as bf16 matmuls (f32 PSUM accumulation),
two 512-wide matmuls per chunk into two PSUM banks; k-chunks 30..31 run
as ONE fp8(e4m3) DoubleRow matmul pair (256-deep contraction, ~1.8x the
bf16 rate).  Putting only 2 of 32 chunks through fp8 keeps the overall
relative error at ~1.2e-2 (measured on the harness distribution)
against the 2e-2 gate while saving ~3% wall clock.  The PSUM->SBUF
bias-add runs on DVE and stores bf16 (error contribution ~1e-3).

The per-token LoRA gate fires on ~1/64 tokens (ids uniform in [0,64)),
so the LoRA path is sparse: the host gathers the masked tokens of each
half into <=256 compact rows (2 strips), the device computes the rank-8
update only for those rows (full-precision bf16 path) into a separate
`lorac` output, and the host scatter-adds that shard back during
unshard.  This keeps the LoRA epilogue out of the 64-strip main loop.

Startup is DMA-bandwidth-bound (W must land while x streams), so the
first 8 strips run CHUNK-MAJOR in two groups of 4: each W chunk is
consumed by 8 matmuls the moment it arrives, which stretches the W-load
deadline from one strip-time (~14us) to ~55us.  All startup DMAs are
deadline-sorted and dealt to the three DMA-capable queues
(gpsimd/sync/scalar) by weighted fair-queuing.  ~120 tiny dummy matmuls
run during the initial DMA wait so the PE's HAM clock-gate is already
at 2.4 GHz when real work starts.
"""

import sys

for _p in ("/opt/trn_rl_repo",):
    if _p not in sys.path:
        sys.path.insert(0, _p)

from contextlib import ExitStack

import numpy as np
import ml_dtypes

import concourse.bass as bass
import concourse.mybir as mybir
import concourse.tile as tile
from concourse import bacc
from concourse.bass import ts
from concourse.bass_utils import run_bass_kernel_spmd

F32 = mybir.dt.float32
BF16 = mybir.dt.bfloat16
F8 = mybir.dt.float8e4
BF = ml_dtypes.bfloat16
F8NP = ml_dtypes.float8_e4m3

B, S, DI, DO = 4, 4096, 4096, 4096
TOK = B * S              # 16384 tokens
NCORES = 8
TH = TOK // 2            # tokens per core (half)        = 8192
DQ = DO // 4             # d_out per core (quarter)      = 1024
P = 128                  # partition / strip size
KC = DI // P             # k-chunks                      = 32
KF = 2                   # trailing k-chunks in fp8 DoubleRow
KCB = KC - KF            # leading k-chunks in bf16      = 30
NSTRIP = TH // P         # token strips per core         = 64
R = 8                    # LoRA rank
NW = 512                 # matmul moving width (1 PSUM bank of f32)
COMP_TOKEN_ID = 7
SCALING = 2.0
NCAP = 256               # compact (masked-token) capacity per half
NCSTRIP = NCAP // P      # compact strips                = 2
GS = 4                   # strips per chunk-major startup group
NGRP = 2                 # startup groups
NDUMMY = 120             # HAM warm-up matmuls
COMPACT_AT = 16          # strip index where the compact path is placed
DR = mybir.MatmulPerfMode.DoubleRow

# bf16 W tile chunk groups: six single-chunk tiles first (fine-grained
# arrival for the startup-critical chunks), then pairs.
W_GROUPS = [[c] for c in range(6)] + [[c, c + 1] for c in range(6, KCB, 2)]
# group-0 x sub-tile chunk ranges (finer early)
X0_SPLITS = [(0, 4), (4, 9), (9, 16), (16, 23), (23, KCB)]
# group-1 x half-strip chunk ranges
X1_SPLITS = [(0, 15), (15, KCB)]


def _build_nc():
    nc = bacc.Bacc(
        "TRN2",
        target_bir_lowering=False,
        debug=False,
        enable_asserts=True,
        num_devices=NCORES,
    )

    xT_d = nc.dram_tensor("xT", [NSTRIP, P, KCB * P], BF16, kind="ExternalInput").ap()
    x8_d = nc.dram_tensor("x8", [NSTRIP, P, KF * P], F8, kind="ExternalInput").ap()
    w_d = nc.dram_tensor("w", [KCB * P, DQ], BF16, kind="ExternalInput").ap()
    w8_d = nc.dram_tensor("w8", [P, KF, DQ], F8, kind="ExternalInput").ap()
    xcT_d = nc.dram_tensor("xcT", [NCSTRIP, P, KC * P], BF16, kind="ExternalInput").ap()
    aT_d = nc.dram_tensor("aT", [KC * P, R], BF16, kind="ExternalInput").ap()
    bT2_d = nc.dram_tensor("bT2", [R, DQ], BF16, kind="ExternalInput").ap()
    bias_d = nc.dram_tensor("biasr", [P, DQ], F32, kind="ExternalInput").ap()
    id_d = nc.dram_tensor("ident", [P, P], BF16, kind="ExternalInput").ap()
    out_d = nc.dram_tensor("out", [TH, DQ], BF16, kind="ExternalOutput").ap()
    lorac_d = nc.dram_tensor("lorac", [NCAP, DQ], F32, kind="ExternalOutput").ap()

    with tile.TileContext(nc) as tc, ExitStack() as ctx:
        consts = ctx.enter_context(tc.tile_pool(name="consts", bufs=1))
        xpool = ctx.enter_context(tc.tile_pool(name="xp", bufs=2))
        opool = ctx.enter_context(tc.tile_pool(name="op", bufs=4))
        cpool = ctx.enter_context(tc.tile_pool(name="cp", bufs=1))
        psum = ctx.enter_context(tc.tile_pool(name="ps", bufs=1, space="PSUM"))

        # ---- HAM warm-up scaffolding (zeros; results discarded) ----
        wz = consts.tile([P, 64], BF16, name="wz", tag="wz")
        nc.vector.memset(wz[:], 0.0)

        # ---- startup DMAs: every item gets a deadline (us after the
        # first matmul); items are dealt deadline-ordered to the three
        # DMA-capable queues by weighted fair-queuing (observed rates
        # under contention: gpsimd ~170GB/s, sync ~95, scalar ~85) ----
        CHUNK_US = 1.73           # chunk-major group: 8 matmuls of N=512
        items = []                # (deadline, bytes, issue_fn)

        w_tiles = []
        chunk_loc = {}
        for gi, grp in enumerate(W_GROUPS):
            wt = consts.tile([P, len(grp), DQ], BF16, name=f"w{gi}", tag=f"w{gi}")
            w_tiles.append(wt)
            for bi, c in enumerate(grp):
                chunk_loc[c] = (gi, bi)

            def issue_w(eng, wt=wt, grp=grp):
                eng.dma_start(
                    wt[:],
                    w_d[grp[0] * P : (grp[-1] + 1) * P, :].rearrange(
                        "(b p) o -> p b o", p=P
                    ),
                )

            items.append((CHUNK_US * grp[0] - 2.0, len(grp) * P * DQ * 2, issue_w))

        def w_ap(c, j):
            gi, bi = chunk_loc[c]
            return w_tiles[gi][:, bi, j * NW : (j + 1) * NW]

        # fp8 W (chunks 30..31), needed at the tail of group 0
        w8t = consts.tile([P, KF, DQ], F8, name="w8t", tag="w8t")
        items.append(
            (CHUNK_US * KCB - 4.0, P * KF * DQ, lambda eng: eng.dma_start(w8t[:], w8_d[:, :, :]))
        )

        # group-0 x: 4 strips x 4 sub-tiles + fp8 tails
        x0 = [[None] * len(X0_SPLITS) for _ in range(GS)]
        for k, (c0, c1) in enumerate(X0_SPLITS):
            for s in range(GS):
                xh = consts.tile(
                    [P, c1 - c0, P], BF16, name=f"xh{s}_{k}", tag=f"xh{s}_{k}"
                )
                x0[s][k] = xh

                def issue_x0(eng, xh=xh, s=s, c0=c0, c1=c1):
                    eng.dma_start(
                        xh[:],
                        xT_d[s][:, c0 * P : c1 * P].rearrange(
                            "p (c t) -> p c t", t=P
                        ),
                    )

                items.append(
                    (CHUNK_US * c0 + 0.43 * s - 1.5, (c1 - c0) * P * P * 2, issue_x0)
                )
        # group-1 x: half-strip tiles (arrive during group 0)
        x1 = [[None] * len(X1_SPLITS) for _ in range(GS)]
        for hh, (c0, c1) in enumerate(X1_SPLITS):
            for i in range(GS):
                xg = consts.tile(
                    [P, c1 - c0, P], BF16, name=f"xg{i}_{hh}", tag=f"xg{i}_{hh}"
                )
                x1[i][hh] = xg

                def issue_x1(eng, xg=xg, i=i, c0=c0, c1=c1):
                    eng.dma_start(
                        xg[:],
                        xT_d[GS + i][:, c0 * P : c1 * P].rearrange(
                            "p (c t) -> p c t", t=P
                        ),
                    )

                items.append(
                    (
                        KC * CHUNK_US + c0 * CHUNK_US + 0.43 * i - 4.0,
                        (c1 - c0) * P * P * 2,
                        issue_x1,
                    )
                )
        # fp8 x tails for the 8 grouped strips
        x8g = [None] * (NGRP * GS)
        for s in range(NGRP * GS):
            x8h = consts.tile([P, KF, P], F8, name=f"x8h{s}", tag=f"x8h{s}")
            x8g[s] = x8h

            def issue_x8(eng, x8h=x8h, s=s):
                eng.dma_start(
                    x8h[:], x8_d[s].rearrange("p (c t) -> p c t", t=P)
                )

            items.append(
                ((s // GS + 1) * KC * CHUNK_US - 6.0, KF * P * P, issue_x8)
            )

        # bias is first needed when group 0 drains
        biast = consts.tile([P, DQ], F32, name="biast", tag="biast")
        items.append(
            (KC * CHUNK_US + 1.0, P * DQ * 4, lambda eng: eng.dma_start(biast[:], bias_d[:, :]))
        )

        # weighted fair deal-out, deadline order
        items.sort(key=lambda it: it[0])
        queues = [(nc.gpsimd, 1.7), (nc.sync, 1.0), (nc.scalar, 0.9)]
        qbytes = [0.0, 0.0, 0.0]
        for _, nbytes, fn in items:
            qi = min(range(3), key=lambda q: qbytes[q] / queues[q][1])
            fn(queues[qi][0])
            qbytes[qi] += nbytes

        # ---- compact-path constants (needed ~230us in) ----
        at = consts.tile([P, KC, R], BF16, name="at", tag="at")
        nc.gpsimd.dma_start(at[:], aT_d[:, :].rearrange("(c p) r -> p c r", p=P))
        bt2 = consts.tile([R, DQ], BF16, name="bt2", tag="bt2")
        nc.gpsimd.dma_start(bt2[:], bT2_d[:, :])
        ident = consts.tile([P, P], BF16, name="ident", tag="ident")
        nc.gpsimd.dma_start(ident[:], id_d[:, :])
        xct = []
        for cs in range(NCSTRIP):
            xc = consts.tile([P, KC, P], BF16, name=f"xc{cs}", tag=f"xc{cs}")
            nc.gpsimd.dma_start(
                xc[:], xcT_d[cs].rearrange("p (c t) -> p c t", t=P)
            )
            xct.append(xc)

        # ---- PSUM slots: 8 banks, named g0..g7.  Groups use all 8;
        # streamed strips alternate pairs (g0,g1)/(g2,g3); the compact
        # path borrows g4..g7 ----
        def bank(tag, shape=(P, NW), dtype=F32):
            return psum.tile(list(shape), dtype, name=tag, tag=tag)

        # ---- HAM warm-up: tiny matmuls on zeros while DMAs land ----
        dps = bank("g6", (64, 64))
        for _ in range(NDUMMY):
            nc.tensor.matmul(
                dps[:], wz[:, 0:64], wz[:], start=True, stop=True,
                skip_group_check=True,
            )

        def dr_tail(x8sl, q0, q1):
            """chunks 30..31 as one fp8 DoubleRow matmul per bank."""
            nc.tensor.matmul(
                q0[:], x8sl, w8t[:, :, 0:NW], start=False, stop=True,
                perf_mode=DR, skip_group_check=True,
            )
            nc.tensor.matmul(
                q1[:], x8sl, w8t[:, :, NW:DQ], start=False, stop=True,
                perf_mode=DR, skip_group_check=True,
            )

        def drain(s, q0, q1, fine=False):
            """bias-add on DVE (bf16 out) + store one strip."""
            ob = opool.tile([P, DQ], BF16, name="ob", tag="ob")
            nsub = 4 if fine else 2
            w = DQ // nsub
            for i in range(nsub):
                sl = slice(i * w, (i + 1) * w)
                q = q0 if i * w < NW else q1
                qoff = (i * w) % NW
                nc.vector.tensor_add(ob[:, sl], q[:, qoff : qoff + w], biast[:, sl])
                nc.sync.dma_start(out_d[ts(s, P), sl], ob[:, sl])

        def compact_path():
            """rank-8 LoRA for the gathered masked tokens only."""
            for cs in range(NCSTRIP):
                xac_ps = bank("g4", (P, R))
                for c in range(KC):
                    nc.tensor.matmul(
                        xac_ps[:], xct[cs][:, c, :], at[:, c, :],
                        start=(c == 0), stop=(c == KC - 1),
                        skip_group_check=True,
                    )
                xac_sb = cpool.tile([P, R], BF16, name="xacsb", tag="xacsb")
                nc.vector.tensor_copy(xac_sb[:], xac_ps[:])
                xacT_ps = bank("g5", (R, P), BF16)
                nc.tensor.transpose(xacT_ps[:], xac_sb[:], ident[:])
                xacT = cpool.tile([R, P], BF16, name="xacTsb", tag="xacTsb")
                nc.vector.tensor_copy(xacT[:], xacT_ps[:])
                lc0 = bank("g6")
                lc1 = bank("g7")
                nc.tensor.matmul(
                    lc0[:], xacT[:], bt2[:, 0:NW], start=True, stop=True,
                    skip_group_check=True,
                )
                nc.tensor.matmul(
                    lc1[:], xacT[:], bt2[:, NW:DQ], start=True, stop=True,
                    skip_group_check=True,
                )
                obc = cpool.tile([P, DQ], F32, name="obc", tag=f"obc{cs}")
                nc.vector.tensor_copy(obc[:, 0:NW], lc0[:])
                nc.vector.tensor_copy(obc[:, NW:DQ], lc1[:])
                nc.gpsimd.dma_start(lorac_d[ts(cs, P), :], obc[:])

        def x0_ap(i, c):
            for k, (c0, c1) in enumerate(X0_SPLITS):
                if c0 <= c < c1:
                    return x0[i][k][:, c - c0, :]

        def x1_ap(i, c):
            for k, (c0, c1) in enumerate(X1_SPLITS):
                if c0 <= c < c1:
                    return x1[i][k][:, c - c0, :]

        # ---- phase 1: two chunk-major groups (strips 0..7) ----
        for g in range(NGRP):
            qs = [
                (bank(f"g{2 * i}"), bank(f"g{2 * i + 1}")) for i in range(GS)
            ]
            for c in range(KCB):
                for i in range(GS):
                    lt = x0_ap(i, c) if g == 0 else x1_ap(i, c)
                    q0, q1 = qs[i]
                    nc.tensor.matmul(
                        q0[:], lt, w_ap(c, 0),
                        start=(c == 0), stop=False, skip_group_check=True,
                    )
                    nc.tensor.matmul(
                        q1[:], lt, w_ap(c, 1),
                        start=(c == 0), stop=False, skip_group_check=True,
                    )
            for i in range(GS):
                dr_tail(x8g[g * GS + i][:, :, :], *qs[i])
            for i in range(GS):
                drain(g * GS + i, *qs[i])

        # ---- phase 2: streamed strips (8..63) ----
        for s in range(NGRP * GS, NSTRIP):
            xt = xpool.tile([P, KCB, P], BF16, name="xt", tag="xt")
            nc.scalar.dma_start(xt[:], xT_d[s].rearrange("p (c t) -> p c t", t=P))
            x8t = xpool.tile([P, KF, P], F8, name="x8t", tag="x8t")
            nc.scalar.dma_start(x8t[:], x8_d[s].rearrange("p (c t) -> p c t", t=P))
            pair = s % 2
            q0 = bank(f"g{2 * pair}")
            q1 = bank(f"g{2 * pair + 1}")
            for c in range(KCB):
                lt = xt[:, c, :]
                nc.tensor.matmul(
                    q0[:], lt, w_ap(c, 0),
                    start=(c == 0), stop=False, skip_group_check=True,
                )
                nc.tensor.matmul(
                    q1[:], lt, w_ap(c, 1),
                    start=(c == 0), stop=False, skip_group_check=True,
                )
            dr_tail(x8t[:, :, :], q0, q1)
            drain(s, q0, q1, fine=(s == NSTRIP - 1))
            if s == COMPACT_AT:
                compact_path()

    nc.compile()
    return nc


_NC_CACHE = None


def _get_nc():
    global _NC_CACHE
    if _NC_CACHE is None:
        _NC_CACHE = _build_nc()
    return _NC_CACHE


def _make_strips(cols, nch):
    # strips[s, p, c*128+t] = cols[c*128+p, s*128+t]
    n = cols.shape[1]
    ns = n // P
    return np.ascontiguousarray(
        cols.reshape(nch, P, ns, P).transpose(2, 1, 0, 3).reshape(ns, P, nch * P)
    )


def _make_in_maps(x, ids, W, b, lora_A, lora_B):
    x2 = np.asarray(x, dtype=np.float32).reshape(TOK, DI)
    xT = np.ascontiguousarray(x2.T)                          # [DI, TOK] f32
    xTb = xT[: KCB * P].astype(BF)                           # bf16 head chunks
    xT8 = xT[KCB * P :].astype(F8NP)                         # fp8 tail chunks
    WT = np.asarray(W, dtype=np.float32).T                   # [DI, DO]
    WTb = WT[: KCB * P].astype(BF)
    WT8 = WT[KCB * P :].astype(F8NP)                         # [KF*P, DO]
    AT = np.asarray(lora_A, dtype=np.float32).T.astype(BF)   # [DI, R]
    BT2 = (np.asarray(lora_B, dtype=np.float32).T * SCALING).astype(BF)  # [R, DO]
    bias = np.asarray(b, dtype=np.float32)
    maskb = np.asarray(ids).reshape(TOK) == COMP_TOKEN_ID

    xT_half, x8_half, xcT_half, idx_half = [], [], [], []
    xTc = xT.astype(BF)                                      # full bf16 (compact)
    for h in range(2):
        sl = slice(h * TH, (h + 1) * TH)
        xT_half.append(_make_strips(xTb[:, sl], KCB))
        x8_half.append(_make_strips(xT8[:, sl], KF))
        idx = np.nonzero(maskb[sl])[0]
        idx_half.append(idx)
        xc = np.zeros((DI, NCAP), dtype=BF)
        ncnt = min(len(idx), NCAP)
        xc[:, :ncnt] = xTc[:, h * TH + idx[:ncnt]]
        xcT_half.append(_make_strips(xc, KC))

    w_q, w8_q, bT2_q, bias_q = [], [], [], []
    for qi in range(4):
        sl = slice(qi * DQ, (qi + 1) * DQ)
        w_q.append(np.ascontiguousarray(WTb[:, sl]))
        # w8[p, i, :] = W[KCB*P + i*P + p, quarter]
        w8_q.append(
            np.ascontiguousarray(WT8[:, sl].reshape(KF, P, DQ).transpose(1, 0, 2))
        )
        bT2_q.append(np.ascontiguousarray(BT2[:, sl]))
        bias_q.append(
            np.ascontiguousarray(np.broadcast_to(bias[sl], (P, DQ)))
        )
    ident = np.eye(P, dtype=np.float32).astype(BF)

    in_maps = []
    for c in range(NCORES):
        h, qi = c // 4, c % 4
        in_maps.append(
            {
                "xT": xT_half[h],
                "x8": x8_half[h],
                "w": w_q[qi],
                "w8": w8_q[qi],
                "xcT": xcT_half[h],
                "aT": AT,
                "bT2": bT2_q[qi],
                "biasr": bias_q[qi],
                "ident": ident,
            }
        )
    return in_maps, idx_half


def kernel(x, ids, W, b, lora_A, lora_B):
    nc = _get_nc()
    in_maps, idx_half = _make_in_maps(x, ids, W, b, lora_A, lora_B)
    results = run_bass_kernel_spmd(nc, in_maps, core_ids=list(range(NCORES)))
    out = np.empty((TOK, DO), dtype=np.float32)
    for c in range(NCORES):
        h, qi = c // 4, c % 4
        res = results.results[c]
        out[h * TH : (h + 1) * TH, qi * DQ : (qi + 1) * DQ] = np.asarray(
            res["out"], dtype=np.float32
        )
        idx = idx_half[h]
        ncnt = min(len(idx), NCAP)
        # unshard: sum-combine the sparse LoRA shard into the dense shard
        out[h * TH + idx[:ncnt], qi * DQ : (qi + 1) * DQ] += res["lorac"][:ncnt]
    if any(len(idx) > NCAP for idx in idx_half):
        # overflow fallback (never hit for the harness distribution):
        # finish the remaining rows' rank-8 update on the host
        xf = np.asarray(x, dtype=np.float32).reshape(TOK, DI)
        Af = np.asarray(lora_A, dtype=np.float32)
        Bf = np.asarray(lora_B, dtype=np.float32)
        for h in range(2):
            extra = idx_half[h][NCAP:]
            if len(extra):
                rows = h * TH + extra
                out[rows] += SCALING * (xf[rows] @ Af.T) @ Bf.T
    return out.reshape(B, S, DO)


if __name__ == "__main__":
    rng = np.random.default_rng(0)
    x = rng.standard_normal((B, S, DI), dtype=np.float32)
    ids = rng.integers(0, 64, size=(B, S)).astype(np.int64)
    W = rng.standard_normal((DO, DI), dtype=np.float32) / np.sqrt(DI)
    b = (rng.standard_normal(DO) * 0.02).astype(np.float32)
    lora_A = rng.standard_normal((8, DI), dtype=np.float32) / np.sqrt(DI)
    lora_B = (rng.standard_normal((DO, 8)) * 0.02).astype(np.float32)
    out = kernel(x, ids, W, b, lora_A, lora_B)
    print(out.shape, out.dtype, float(np.abs(out).mean()))


# revision 16
# speedup vs baseline: 1.0110x; 1.0110x over previous
"""Trainium2 Bass kernel for ConditionalLoRALinear.

Reference computation (f32):
    base = x @ W.T + b                      # [B,S,Do]
    lora = (x @ A.T) @ B.T * 2.0            # rank-8
    out  = base + lora * (ids == 7)         # per-token gate

Sharding over 8 NeuronCores: 2 token-halves x 4 d_out-quarters.

Main loop: k-chunks 0..29 run as bf16 matmuls (f32 PSUM accumulation),
two 512-wide matmuls per chunk into two PSUM banks; k-chunks 30..31 run
as ONE fp8(e4m3) DoubleRow matmul pair (256-deep contraction, ~1.8x the
bf16 rate).  Putting only 2 of 32 chunks through fp8 keeps the overall
relative error at ~1.2e-2 (measured on the harness distribution)
against the 2e-2 gate while saving ~3% wall clock.  The PSUM->SBUF
bias-add runs on DVE and stores bf16 (error contribution ~1e-3).

The per-token LoRA gate fires on ~1/64 tokens (ids uniform in [0,64)),
so the LoRA path is sparse: the host gathers the masked tokens of each
half into <=256 compact rows (2 strips), the device computes the rank-8
update only for those rows (full-precision bf16 path) into a separate
`lorac` output, and the host scatter-adds that shard back during
unshard.  This keeps the LoRA epilogue out of the 64-strip main loop.

Startup is DMA-bandwidth-bound (W must land while x streams), so the
first 8 strips run CHUNK-MAJOR in two groups of 4: each W chunk is
consumed by 8 matmuls the moment it arrives, which stretches the W-load
deadline from one strip-time (~14us) to ~55us.  All startup DMAs are
deadline-sorted and dealt to the three DMA-capable queues
(gpsimd/sync/scalar) by weighted fair-queuing.  ~120 tiny dummy matmuls
run during the initial DMA wait so the PE's HAM clock-gate is already
at 2.4 GHz when real work starts.
"""

import sys

for _p in ("/opt/trn_rl_repo",):
    if _p not in sys.path:
        sys.path.insert(0, _p)

from contextlib import ExitStack

import numpy as np
import ml_dtypes

import concourse.bass as bass
import concourse.mybir as mybir
import concourse.tile as tile
from concourse import bacc
from concourse.bass import ts
from concourse.bass_utils import run_bass_kernel_spmd

F32 = mybir.dt.float32
BF16 = mybir.dt.bfloat16
F8 = mybir.dt.float8e4
BF = ml_dtypes.bfloat16
F8NP = ml_dtypes.float8_e4m3

B, S, DI, DO = 4, 4096, 4096, 4096
TOK = B * S              # 16384 tokens
NCORES = 8
TH = TOK // 2            # tokens per core (half)        = 8192
DQ = DO // 4             # d_out per core (quarter)      = 1024
P = 128                  # partition / strip size
KC = DI // P             # k-chunks                      = 32
KF = 2                   # trailing k-chunks in fp8 DoubleRow
KCB = KC - KF            # leading k-chunks in bf16      = 30
NSTRIP = TH // P         # token strips per core         = 64
R = 8                    # LoRA rank
NW = 512                 # matmul moving width (1 PSUM bank of f32)
COMP_TOKEN_ID = 7
SCALING = 2.0
NCAP = 256               # compact (masked-token) capacity per half
NCSTRIP = NCAP // P      # compact strips                = 2
GS = 4                   # strips per chunk-major startup group
NGRP = 2                 # startup groups
NDUMMY = 120             # HAM warm-up matmuls
COMPACT_AT = 16          # strip index where the compact path is placed
DR = mybir.MatmulPerfMode.DoubleRow

# bf16 W tile chunk groups: two single-chunk tiles first (so the first
# matmuls only wait on a 256KB transfer), then pairs.
W_GROUPS = [[0], [1]] + [[c, c + 1] for c in range(2, KCB, 2)]
# group-0 x sub-tile chunk ranges
X0_SPLITS = [(0, 8), (8, 16), (16, 24), (24, KCB)]
# group-1 x half-strip chunk ranges
X1_SPLITS = [(0, 15), (15, KCB)]


def _build_nc():
    nc = bacc.Bacc(
        "TRN2",
        target_bir_lowering=False,
        debug=False,
        enable_asserts=True,
        num_devices=NCORES,
    )

    xT_d = nc.dram_tensor("xT", [NSTRIP, P, KCB * P], BF16, kind="ExternalInput").ap()
    x8_d = nc.dram_tensor("x8", [NSTRIP, P, KF * P], F8, kind="ExternalInput").ap()
    w_d = nc.dram_tensor("w", [KCB * P, DQ], BF16, kind="ExternalInput").ap()
    w8_d = nc.dram_tensor("w8", [P, KF, DQ], F8, kind="ExternalInput").ap()
    xcT_d = nc.dram_tensor("xcT", [NCSTRIP, P, KC * P], BF16, kind="ExternalInput").ap()
    aT_d = nc.dram_tensor("aT", [KC * P, R], BF16, kind="ExternalInput").ap()
    bT2_d = nc.dram_tensor("bT2", [R, DQ], BF16, kind="ExternalInput").ap()
    bias_d = nc.dram_tensor("biasr", [P, DQ], F32, kind="ExternalInput").ap()
    id_d = nc.dram_tensor("ident", [P, P], BF16, kind="ExternalInput").ap()
    out_d = nc.dram_tensor("out", [TH, DQ], BF16, kind="ExternalOutput").ap()
    lorac_d = nc.dram_tensor("lorac", [NCAP, DQ], F32, kind="ExternalOutput").ap()

    with tile.TileContext(nc) as tc, ExitStack() as ctx:
        consts = ctx.enter_context(tc.tile_pool(name="consts", bufs=1))
        xpool = ctx.enter_context(tc.tile_pool(name="xp", bufs=2))
        opool = ctx.enter_context(tc.tile_pool(name="op", bufs=4))
        cpool = ctx.enter_context(tc.tile_pool(name="cp", bufs=1))
        psum = ctx.enter_context(tc.tile_pool(name="ps", bufs=1, space="PSUM"))

        # ---- HAM warm-up scaffolding (zeros; results discarded) ----
        wz = consts.tile([P, 64], BF16, name="wz", tag="wz")
        nc.vector.memset(wz[:], 0.0)

        # ---- startup DMAs: every item gets a deadline (us after the
        # first matmul); items are dealt deadline-ordered to the three
        # DMA-capable queues by weighted fair-queuing (observed rates
        # under contention: gpsimd ~170GB/s, sync ~95, scalar ~85) ----
        CHUNK_US = 1.73           # chunk-major group: 8 matmuls of N=512
        items = []                # (deadline, bytes, issue_fn)

        w_tiles = []
        chunk_loc = {}
        for gi, grp in enumerate(W_GROUPS):
            wt = consts.tile([P, len(grp), DQ], BF16, name=f"w{gi}", tag=f"w{gi}")
            w_tiles.append(wt)
            for bi, c in enumerate(grp):
                chunk_loc[c] = (gi, bi)

            def issue_w(eng, wt=wt, grp=grp):
                eng.dma_start(
                    wt[:],
                    w_d[grp[0] * P : (grp[-1] + 1) * P, :].rearrange(
                        "(b p) o -> p b o", p=P
                    ),
                )

            items.append((CHUNK_US * grp[0] - 2.0, len(grp) * P * DQ * 2, issue_w))

        def w_ap(c, j):
            gi, bi = chunk_loc[c]
            return w_tiles[gi][:, bi, j * NW : (j + 1) * NW]

        # fp8 W (chunks 30..31), needed at the tail of group 0
        w8t = consts.tile([P, KF, DQ], F8, name="w8t", tag="w8t")
        items.append(
            (CHUNK_US * KCB - 4.0, P * KF * DQ, lambda eng: eng.dma_start(w8t[:], w8_d[:, :, :]))
        )

        # group-0 x: 4 strips x 4 sub-tiles + fp8 tails
        x0 = [[None] * len(X0_SPLITS) for _ in range(GS)]
        for k, (c0, c1) in enumerate(X0_SPLITS):
            for s in range(GS):
                xh = consts.tile(
                    [P, c1 - c0, P], BF16, name=f"xh{s}_{k}", tag=f"xh{s}_{k}"
                )
                x0[s][k] = xh

                def issue_x0(eng, xh=xh, s=s, c0=c0, c1=c1):
                    eng.dma_start(
                        xh[:],
                        xT_d[s][:, c0 * P : c1 * P].rearrange(
                            "p (c t) -> p c t", t=P
                        ),
                    )

                items.append(
                    (CHUNK_US * c0 + 0.43 * s - 1.5, (c1 - c0) * P * P * 2, issue_x0)
                )
        # group-1 x: half-strip tiles (arrive during group 0)
        x1 = [[None] * len(X1_SPLITS) for _ in range(GS)]
        for hh, (c0, c1) in enumerate(X1_SPLITS):
            for i in range(GS):
                xg = consts.tile(
                    [P, c1 - c0, P], BF16, name=f"xg{i}_{hh}", tag=f"xg{i}_{hh}"
                )
                x1[i][hh] = xg

                def issue_x1(eng, xg=xg, i=i, c0=c0, c1=c1):
                    eng.dma_start(
                        xg[:],
                        xT_d[GS + i][:, c0 * P : c1 * P].rearrange(
                            "p (c t) -> p c t", t=P
                        ),
                    )

                items.append(
                    (
                        KC * CHUNK_US + c0 * CHUNK_US + 0.43 * i - 4.0,
                        (c1 - c0) * P * P * 2,
                        issue_x1,
                    )
                )
        # fp8 x tails for the 8 grouped strips
        x8g = [None] * (NGRP * GS)
        for s in range(NGRP * GS):
            x8h = consts.tile([P, KF, P], F8, name=f"x8h{s}", tag=f"x8h{s}")
            x8g[s] = x8h

            def issue_x8(eng, x8h=x8h, s=s):
                eng.dma_start(
                    x8h[:], x8_d[s].rearrange("p (c t) -> p c t", t=P)
                )

            items.append(
                ((s // GS + 1) * KC * CHUNK_US - 6.0, KF * P * P, issue_x8)
            )

        # bias is first needed when group 0 drains
        biast = consts.tile([P, DQ], F32, name="biast", tag="biast")
        items.append(
            (KC * CHUNK_US + 1.0, P * DQ * 4, lambda eng: eng.dma_start(biast[:], bias_d[:, :]))
        )

        # weighted fair deal-out, deadline order
        items.sort(key=lambda it: it[0])
        queues = [(nc.gpsimd, 1.7), (nc.sync, 1.0), (nc.scalar, 0.9)]
        qbytes = [0.0, 0.0, 0.0]
        for _, nbytes, fn in items:
            qi = min(range(3), key=lambda q: qbytes[q] / queues[q][1])
            fn(queues[qi][0])
            qbytes[qi] += nbytes

        # ---- compact-path constants (needed ~230us in) ----
        at = consts.tile([P, KC, R], BF16, name="at", tag="at")
        nc.gpsimd.dma_start(at[:], aT_d[:, :].rearrange("(c p) r -> p c r", p=P))
        bt2 = consts.tile([R, DQ], BF16, name="bt2", tag="bt2")
        nc.gpsimd.dma_start(bt2[:], bT2_d[:, :])
        ident = consts.tile([P, P], BF16, name="ident", tag="ident")
        nc.gpsimd.dma_start(ident[:], id_d[:, :])
        xct = []
        for cs in range(NCSTRIP):
            xc = consts.tile([P, KC, P], BF16, name=f"xc{cs}", tag=f"xc{cs}")
            nc.gpsimd.dma_start(
                xc[:], xcT_d[cs].rearrange("p (c t) -> p c t", t=P)
            )
            xct.append(xc)

        # ---- PSUM slots: 8 banks, named g0..g7.  Groups use all 8;
        # streamed strips alternate pairs (g0,g1)/(g2,g3); the compact
        # path borrows g4..g7 ----
        def bank(tag, shape=(P, NW), dtype=F32):
            return psum.tile(list(shape), dtype, name=tag, tag=tag)

        # ---- HAM warm-up: tiny matmuls on zeros while DMAs land ----
        dps = bank("g6", (64, 64))
        for _ in range(NDUMMY):
            nc.tensor.matmul(
                dps[:], wz[:, 0:64], wz[:], start=True, stop=True,
                skip_group_check=True,
            )

        def dr_tail(x8sl, q0, q1):
            """chunks 30..31 as one fp8 DoubleRow matmul per bank."""
            nc.tensor.matmul(
                q0[:], x8sl, w8t[:, :, 0:NW], start=False, stop=True,
                perf_mode=DR, skip_group_check=True,
            )
            nc.tensor.matmul(
                q1[:], x8sl, w8t[:, :, NW:DQ], start=False, stop=True,
                perf_mode=DR, skip_group_check=True,
            )

        def drain(s, q0, q1, fine=False):
            """bias-add on DVE (bf16 out) + store one strip."""
            ob = opool.tile([P, DQ], BF16, name="ob", tag="ob")
            nsub = 4 if fine else 2
            w = DQ // nsub
            for i in range(nsub):
                sl = slice(i * w, (i + 1) * w)
                q = q0 if i * w < NW else q1
                qoff = (i * w) % NW
                nc.vector.tensor_add(ob[:, sl], q[:, qoff : qoff + w], biast[:, sl])
                nc.sync.dma_start(out_d[ts(s, P), sl], ob[:, sl])

        def compact_path():
            """rank-8 LoRA for the gathered masked tokens only."""
            for cs in range(NCSTRIP):
                xac_ps = bank("g4", (P, R))
                for c in range(KC):
                    nc.tensor.matmul(
                        xac_ps[:], xct[cs][:, c, :], at[:, c, :],
                        start=(c == 0), stop=(c == KC - 1),
                        skip_group_check=True,
                    )
                xac_sb = cpool.tile([P, R], BF16, name="xacsb", tag="xacsb")
                nc.vector.tensor_copy(xac_sb[:], xac_ps[:])
                xacT_ps = bank("g5", (R, P), BF16)
                nc.tensor.transpose(xacT_ps[:], xac_sb[:], ident[:])
                xacT = cpool.tile([R, P], BF16, name="xacTsb", tag="xacTsb")
                nc.vector.tensor_copy(xacT[:], xacT_ps[:])
                lc0 = bank("g6")
                lc1 = bank("g7")
                nc.tensor.matmul(
                    lc0[:], xacT[:], bt2[:, 0:NW], start=True, stop=True,
                    skip_group_check=True,
                )
                nc.tensor.matmul(
                    lc1[:], xacT[:], bt2[:, NW:DQ], start=True, stop=True,
                    skip_group_check=True,
                )
                obc = cpool.tile([P, DQ], F32, name="obc", tag=f"obc{cs}")
                nc.vector.tensor_copy(obc[:, 0:NW], lc0[:])
                nc.vector.tensor_copy(obc[:, NW:DQ], lc1[:])
                nc.gpsimd.dma_start(lorac_d[ts(cs, P), :], obc[:])

        def x0_ap(i, c):
            for k, (c0, c1) in enumerate(X0_SPLITS):
                if c0 <= c < c1:
                    return x0[i][k][:, c - c0, :]

        def x1_ap(i, c):
            for k, (c0, c1) in enumerate(X1_SPLITS):
                if c0 <= c < c1:
                    return x1[i][k][:, c - c0, :]

        # ---- phase 1: two chunk-major groups (strips 0..7) ----
        for g in range(NGRP):
            qs = [
                (bank(f"g{2 * i}"), bank(f"g{2 * i + 1}")) for i in range(GS)
            ]
            for c in range(KCB):
                for i in range(GS):
                    lt = x0_ap(i, c) if g == 0 else x1_ap(i, c)
                    q0, q1 = qs[i]
                    nc.tensor.matmul(
                        q0[:], lt, w_ap(c, 0),
                        start=(c == 0), stop=False, skip_group_check=True,
                    )
                    nc.tensor.matmul(
                        q1[:], lt, w_ap(c, 1),
                        start=(c == 0), stop=False, skip_group_check=True,
                    )
            for i in range(GS):
                dr_tail(x8g[g * GS + i][:, :, :], *qs[i])
            for i in range(GS):
                drain(g * GS + i, *qs[i])

        # ---- phase 2: streamed strips (8..63) ----
        for s in range(NGRP * GS, NSTRIP):
            xt = xpool.tile([P, KCB, P], BF16, name="xt", tag="xt")
            nc.scalar.dma_start(xt[:], xT_d[s].rearrange("p (c t) -> p c t", t=P))
            x8t = xpool.tile([P, KF, P], F8, name="x8t", tag="x8t")
            nc.scalar.dma_start(x8t[:], x8_d[s].rearrange("p (c t) -> p c t", t=P))
            pair = s % 2
            q0 = bank(f"g{2 * pair}")
            q1 = bank(f"g{2 * pair + 1}")
            for c in range(KCB):
                lt = xt[:, c, :]
                nc.tensor.matmul(
                    q0[:], lt, w_ap(c, 0),
                    start=(c == 0), stop=False, skip_group_check=True,
                )
                nc.tensor.matmul(
                    q1[:], lt, w_ap(c, 1),
                    start=(c == 0), stop=False, skip_group_check=True,
                )
            dr_tail(x8t[:, :, :], q0, q1)
            drain(s, q0, q1, fine=(s == NSTRIP - 1))
            if s == COMPACT_AT:
                compact_path()

    nc.compile()
    return nc


_NC_CACHE = None


def _get_nc():
    global _NC_CACHE
    if _NC_CACHE is None:
        _NC_CACHE = _build_nc()
    return _NC_CACHE


def _make_strips(cols, nch):
    # strips[s, p, c*128+t] = cols[c*128+p, s*128+t]
    n = cols.shape[1]
    ns = n // P
    return np.ascontiguousarray(
        cols.reshape(nch, P, ns, P).transpose(2, 1, 0, 3).reshape(ns, P, nch * P)
    )


def _make_in_maps(x, ids, W, b, lora_A, lora_B):
    x2 = np.asarray(x, dtype=np.float32).reshape(TOK, DI)
    xT = np.ascontiguousarray(x2.T)                          # [DI, TOK] f32
    xTb = xT[: KCB * P].astype(BF)                           # bf16 head chunks
    xT8 = xT[KCB * P :].astype(F8NP)                         # fp8 tail chunks
    WT = np.asarray(W, dtype=np.float32).T                   # [DI, DO]
    WTb = WT[: KCB * P].astype(BF)
    WT8 = WT[KCB * P :].astype(F8NP)                         # [KF*P, DO]
    AT = np.asarray(lora_A, dtype=np.float32).T.astype(BF)   # [DI, R]
    BT2 = (np.asarray(lora_B, dtype=np.float32).T * SCALING).astype(BF)  # [R, DO]
    bias = np.asarray(b, dtype=np.float32)
    maskb = np.asarray(ids).reshape(TOK) == COMP_TOKEN_ID

    xT_half, x8_half, xcT_half, idx_half = [], [], [], []
    xTc = xT.astype(BF)                                      # full bf16 (compact)
    for h in range(2):
        sl = slice(h * TH, (h + 1) * TH)
        xT_half.append(_make_strips(xTb[:, sl], KCB))
        x8_half.append(_make_strips(xT8[:, sl], KF))
        idx = np.nonzero(maskb[sl])[0]
        idx_half.append(idx)
        xc = np.zeros((DI, NCAP), dtype=BF)
        ncnt = min(len(idx), NCAP)
        xc[:, :ncnt] = xTc[:, h * TH + idx[:ncnt]]
        xcT_half.append(_make_strips(xc, KC))

    w_q, w8_q, bT2_q, bias_q = [], [], [], []
    for qi in range(4):
        sl = slice(qi * DQ, (qi + 1) * DQ)
        w_q.append(np.ascontiguousarray(WTb[:, sl]))
        # w8[p, i, :] = W[KCB*P + i*P + p, quarter]
        w8_q.append(
            np.ascontiguousarray(WT8[:, sl].reshape(KF, P, DQ).transpose(1, 0, 2))
        )
        bT2_q.append(np.ascontiguousarray(BT2[:, sl]))
        bias_q.append(
            np.ascontiguousarray(np.broadcast_to(bias[sl], (P, DQ)))
        )
    ident = np.eye(P, dtype=np.float32).astype(BF)

    in_maps = []
    for c in range(NCORES):
        h, qi = c // 4, c % 4
        in_maps.append(
            {
                "xT": xT_half[h],
                "x8": x8_half[h],
                "w": w_q[qi],
                "w8": w8_q[qi],
                "xcT": xcT_half[h],
                "aT": AT,
                "bT2": bT2_q[qi],
                "biasr": bias_q[qi],
                "ident": ident,
            }
        )
    return in_maps, idx_half


def kernel(x, ids, W, b, lora_A, lora_B):
    nc = _get_nc()
    in_maps, idx_half = _make_in_maps(x, ids, W, b, lora_A, lora_B)
    results = run_bass_kernel_spmd(nc, in_maps, core_ids=list(range(NCORES)))
    out = np.empty((TOK, DO), dtype=np.float32)
    for c in range(NCORES):
        h, qi = c // 4, c % 4
        res = results.results[c]
        out[h * TH : (h + 1) * TH, qi * DQ : (qi + 1) * DQ] = np.asarray(
            res["out"], dtype=np.float32
        )
        idx = idx_half[h]
        ncnt = min(len(idx), NCAP)
        # unshard: sum-combine the sparse LoRA shard into the dense shard
        out[h * TH + idx[:ncnt], qi * DQ : (qi + 1) * DQ] += res["lorac"][:ncnt]
    if any(len(idx) > NCAP for idx in idx_half):
        # overflow fallback (never hit for the harness distribution):
        # finish the remaining rows' rank-8 update on the host
        xf = np.asarray(x, dtype=np.float32).reshape(TOK, DI)
        Af = np.asarray(lora_A, dtype=np.float32)
        Bf = np.asarray(lora_B, dtype=np.float32)
        for h in range(2):
            extra = idx_half[h][NCAP:]
            if len(extra):
                rows = h * TH + extra
                out[rows] += SCALING * (xf[rows] @ Af.T) @ Bf.T
    return out.reshape(B, S, DO)


if __name__ == "__main__":
    rng = np.random.default_rng(0)
    x = rng.standard_normal((B, S, DI), dtype=np.float32)
    ids = rng.integers(0, 64, size=(B, S)).astype(np.int64)
    W = rng.standard_normal((DO, DI), dtype=np.float32) / np.sqrt(DI)
    b = (rng.standard_normal(DO) * 0.02).astype(np.float32)
    lora_A = rng.standard_normal((8, DI), dtype=np.float32) / np.sqrt(DI)
    lora_B = (rng.standard_normal((DO, 8)) * 0.02).astype(np.float32)
    out = kernel(x, ids, W, b, lora_A, lora_B)
    print(out.shape, out.dtype, float(np.abs(out).mean()))
